# revision 1
# baseline (speedup 1.0000x reference)
"""Trainium2 Bass kernel for MetaBayesLinearParallel.

Math (per sample s):
    W[s]  = weight_mu + weight_sigma * eps_w[s]          # (OUT, IN)
    Bv[s] = bias_mu + bias_sigma * eps_b[s]              # (OUT,)
    out[s] = x[s] @ W[s].T + Bv[s]                       # (B, OUT)

Sharding over 8 cores: 2-way split of the samples axis x 4-way split of
OUT.  Each core handles S_PC=4 samples and O_PC=512 output rows, which
minimizes per-core HBM traffic (16MB eps + 8MB x + 8MB mu/sigma = 32MB).

Per-core pipeline (all compute in bf16, fp32 PSUM accumulation):
  inputs host-cast to bf16 and host-repacked partition-major, loaded on
  the HWDGE sync ring as 128 contiguous 16KB runs per tensor (16MB/core
  total); bias loads and output stores ride the separate scalar HWDGE
  ring so they never queue behind input loads in the FIFO.
  W built in NATURAL layout on DVE (both ops SBUF+SBUF, 2x mode):
      w = eps; w *= sigma; w += mu        (in-place in the eps tile)
  W-transposes batched per span (4 i-blocks): 16 PE transposes into one
  2-bank PSUM tile, ONE ACT copy -> SBUF, then 8 matmuls; this cuts
  cross-engine semaphore waits ~4x vs per-i-block round trips.
  Steady-state body (on-device For_i loop timing): ~77us/exec.
  Measured alternatives, all slower: fp32 SWDGE cast-DMA 93us, bf16 via
  SWDGE 114us, strided (a p) layout 131-152us, DMA-xbar xT loads 122us,
  x/mu/sigma loads split onto the scalar ring 153us.
"""

from contextlib import ExitStack

import numpy as np

import concourse.bacc as bacc
import concourse.mybir as mybir
import concourse.tile as tile
from concourse.bass_utils import run_bass_kernel_spmd
from concourse.masks import make_identity

P = 128
S, B, IN, OUT = 8, 256, 2048, 2048
SAMPLE_WAYS, OUT_WAYS = 2, 4
N_CORES = SAMPLE_WAYS * OUT_WAYS
S_PC = S // SAMPLE_WAYS
O_PC = OUT // OUT_WAYS

BF16 = mybir.dt.bfloat16
F32 = mybir.dt.float32


def build_core_program(s_pc=S_PC, o_pc=O_PC, in_dim=IN, b_dim=B, repeat=1,
                       skip_input_dma=False, loop_repeat=0):
    """One NeuronCore's program; identical on all cores (SPMD over slices)."""
    o_tiles = o_pc // P
    i_blks = in_dim // P
    b_tiles = b_dim // P
    i_spans = 4
    span = in_dim // i_spans          # 512
    ibs_per_span = i_blks // i_spans  # 4

    nc = bacc.Bacc("TRN2")
    # Inputs are host-repacked to partition-major [P, a, i] so each load is
    # 128 fully-contiguous 32KB runs (4x fewer, 4x fatter descriptors than
    # the strided "(a p) i" layout; lifts the input stream from 245 GB/s to
    # ~line rate).  fp32 source + SWDGE fp32->bf16 cast-DMA measured FASTER
    # than uploading bf16 directly (93us vs 114us full-kernel body).
    x_d = nc.declare_dram_parameter("x", [s_pc, P, b_dim // P, in_dim], BF16,
                                    isOutput=False)
    eps_d = nc.declare_dram_parameter("eps_w", [s_pc, P, o_pc // P, in_dim], BF16,
                                      isOutput=False)
    mu_d = nc.declare_dram_parameter("mu", [P, o_pc // P, in_dim], BF16, isOutput=False)
    sig_d = nc.declare_dram_parameter("sigma", [P, o_pc // P, in_dim], BF16,
                                      isOutput=False)
    bmu_d = nc.declare_dram_parameter("bias_mu", [1, o_pc], F32, isOutput=False)
    bsig_d = nc.declare_dram_parameter("bias_sigma", [1, o_pc], F32, isOutput=False)
    epsb_d = nc.declare_dram_parameter("eps_b", [s_pc, o_pc], F32, isOutput=False)
    out_d = nc.declare_dram_parameter("out", [s_pc, b_dim, o_pc], F32, isOutput=True)

    with ExitStack() as ctx:
        tc = ctx.enter_context(tile.TileContext(nc))
        consts = ctx.enter_context(tc.tile_pool(name="consts", bufs=1))
        resident = ctx.enter_context(tc.tile_pool(name="resident", bufs=1))
        ld = ctx.enter_context(tc.tile_pool(name="ld", bufs=4))
        eps_pool = ctx.enter_context(tc.tile_pool(name="eps_pool", bufs=4))
        xb_pool = ctx.enter_context(tc.tile_pool(name="xb_pool", bufs=2))
        wt_pool = ctx.enter_context(tc.tile_pool(name="wt", bufs=3))
        outp = ctx.enter_context(tc.tile_pool(name="outp", bufs=4))
        ps_tr = ctx.enter_context(tc.tile_pool(name="ps_tr", bufs=2, space="PSUM"))
        ps_xt = ctx.enter_context(tc.tile_pool(name="ps_xt", bufs=1, space="PSUM"))
        ps_out = ctx.enter_context(tc.tile_pool(name="ps_out", bufs=3, space="PSUM"))

        ident = consts.tile([P, P], BF16)
        make_identity(nc, ident)
        ones = consts.tile([1, P], BF16)
        nc.vector.memset(ones[:], 1.0)

        args = (nc, tc, consts, resident, ld, eps_pool, xb_pool,
                wt_pool, outp, ps_tr, ps_xt, ps_out, ident, ones,
                x_d, eps_d, mu_d, sig_d, bmu_d, bsig_d, epsb_d, out_d,
                s_pc, o_pc, in_dim, b_dim, o_tiles, i_blks, b_tiles,
                i_spans, span, ibs_per_span)
        if loop_repeat:
            with tc.For_i(0, loop_repeat, 1):
                _kernel_body(*args, 0, skip_input_dma)
        else:
            for rep in range(repeat):
                _kernel_body(*args, rep, skip_input_dma)

    nc.compile()
    return nc


def _kernel_body(nc, tc, consts, resident, ld, eps_pool, xb_pool, wt_pool,
                 outp, ps_tr, ps_xt, ps_out, ident, ones,
                 x_d, eps_d, mu_d, sig_d, bmu_d, bsig_d, epsb_d, out_d,
                 s_pc, o_pc, in_dim, b_dim, o_tiles, i_blks, b_tiles,
                 i_spans, span, ibs_per_span, rep, skip_input_dma):
    def in_dma(out, in_):
        if not skip_input_dma:
            nc.sync.dma_start(out=out, in_=in_)
        else:
            nc.gpsimd.memset(out, 0.25)

    # ---------------- input DMA issue order (SWDGE queue is FIFO) ---------
    xb_tiles = []
    eps_tiles = {}

    def load_x(s):
        xb = xb_pool.tile([P, b_tiles, in_dim], BF16, tag="xb", name=f"xb_{rep}_{s}")
        in_dma(xb[:], x_d[s])
        xb_tiles.append(xb)

    def load_eps(s):
        # ONE fat dma_start per sample, fully contiguous per partition.
        eps_tiles[s] = eps_pool.tile([P, o_tiles, in_dim], BF16, tag="eps_ld",
                                     name=f"eps_{rep}_{s}")
        in_dma(eps_tiles[s][:], eps_d[s])

    sigma_sb = resident.tile([P, o_tiles, in_dim], BF16, tag="sigma", name=f"sigma_{rep}")
    mu_sb = resident.tile([P, o_tiles, in_dim], BF16, tag="mu", name=f"mu_{rep}")

    load_x(0)
    in_dma(sigma_sb[:], sig_d[:])
    in_dma(mu_sb[:], mu_d[:])
    load_eps(0)
    if s_pc > 1:
        load_x(1)  # early: sample 1's xT builds mid-sample-0
    for s in range(1, s_pc):
        if s > 1:
            load_x(s)
        load_eps(s)

    # bias inputs (tiny, HWDGE)
    bmu_sb = consts.tile([1, o_pc], F32, tag="bmu", name=f"bmu_{rep}")
    nc.scalar.dma_start(out=bmu_sb[:], in_=bmu_d[:, :])
    bsig_sb = consts.tile([1, o_pc], F32, tag="bsig", name=f"bsig_{rep}")
    nc.scalar.dma_start(out=bsig_sb[:], in_=bsig_d[:, :])
    epsb_sb = consts.tile([1, s_pc * o_pc], F32, tag="epsb", name=f"epsb_{rep}")
    nc.scalar.dma_start(out=epsb_sb[:], in_=epsb_d[:, :])

    # ---------------- xT builder (2 i-blocks per PSUM round trip) ---------
    xT_all = resident.tile([P, s_pc, i_blks, b_dim], BF16, tag="xT", name=f"xT_{rep}")

    def build_xT(s, ib_lo, ib_hi):
        for g in range(ib_lo, ib_hi, 2):
            pxt = ps_xt.tile([P, 2, b_dim], BF16, tag="ps_xt", name=f"pxt_{rep}_{s}_{g}")
            for j in range(2):
                for bt in range(b_tiles):
                    nc.tensor.transpose(
                        pxt[:, j, bt * P:(bt + 1) * P],
                        xb_tiles[s][:, bt, (g + j) * P:(g + j + 1) * P], ident[:])
            nc.vector.tensor_copy(xT_all[:, s, g:g + 2, :], pxt[:])

    # ---------------- per-sample compute ---------------------------------
    bv_tiles = {}
    built_spans = set()
    bias_done = set()

    def make_bias(s):
        if s in bias_done:
            return
        bias_done.add(s)
        btmp = ld.tile([1, o_pc], F32, tag="btmp")
        nc.vector.tensor_mul(btmp[:], bsig_sb[:], epsb_sb[:, s * o_pc:(s + 1) * o_pc])
        bv = ld.tile([1, o_pc], BF16, tag="bv", name=f"bv_{rep}_{s}")
        nc.vector.tensor_add(bv[:], bmu_sb[:], btmp[:])
        bv_tiles[s] = bv

    def ensure_w_span(s, isp):
        # in-place: eps tile becomes W = mu + sigma*eps (natural layout)
        if (s, isp) in built_spans:
            return
        built_spans.add((s, isp))
        sl = slice(isp * span, (isp + 1) * span)
        w = eps_tiles[s]
        nc.vector.tensor_mul(w[:, :, sl], w[:, :, sl], sigma_sb[:, :, sl])
        nc.vector.tensor_add(w[:, :, sl], w[:, :, sl], mu_sb[:, :, sl])

    def tr_span(s, isp):
        # W^T for one span (4 i-blocks): 16 PE transposes -> one 2-bank
        # PSUM tile -> ONE ACT copy to SBUF.
        ensure_w_span(s, isp)
        w = eps_tiles[s]
        pwT = ps_tr.tile([P, ibs_per_span, o_pc], BF16, tag="ps_wT",
                         name=f"pwT_{rep}_{s}_{isp}")
        for j in range(ibs_per_span):
            ib = isp * ibs_per_span + j
            for ot in range(o_tiles):
                nc.tensor.transpose(
                    pwT[:, j, ot * P:(ot + 1) * P],
                    w[:, ot, ib * P:(ib + 1) * P], ident[:])
        wtsp = wt_pool.tile([P, ibs_per_span, o_pc], BF16, tag="wt")
        nc.scalar.copy(wtsp[:], pwT[:])
        return wtsp

    # prologue: xT for sample 0, first span staged
    build_xT(0, 0, i_blks)
    make_bias(0)
    wt_q = [tr_span(0, 0)]

    for s in range(s_pc):
        psum_out = []
        for bt in range(b_tiles):
            po = ps_out.tile([P, o_pc], F32, tag="ps_out", name=f"ps_out_{rep}_{s}_{bt}")
            psum_out.append(po)

        for isp in range(i_spans):
            # stage the next span (PE transposes + ACT copy run while this
            # span's matmuls stream)
            if isp + 1 < i_spans:
                ensure_w_span(s, isp + 2) if isp + 2 < i_spans else None
                wt_q.append(tr_span(s, isp + 1))
            elif s + 1 < s_pc:
                make_bias(s + 1)
                wt_q.append(tr_span(s + 1, 0))
            # next sample's xT built in the back half of this sample
            if s + 1 < s_pc and isp >= i_spans - 2:
                h = i_blks // 2
                q = (isp - (i_spans - 2)) * h
                build_xT(s + 1, q, q + h)
            wtsp = wt_q.pop(0)
            for j in range(ibs_per_span):
                ib = isp * ibs_per_span + j
                for bt in range(b_tiles):
                    nc.tensor.matmul(
                        psum_out[bt][:], xT_all[:, s, ib, bt * P:(bt + 1) * P],
                        wtsp[:, j, :], start=(ib == 0), stop=False)
        for bt in range(b_tiles):
            nc.tensor.matmul(psum_out[bt][:], ones[:], bv_tiles[s][:],
                             start=False, stop=True)
            o_sb = outp.tile([P, o_pc], F32, tag="o_sb")
            nc.scalar.copy(o_sb[:], psum_out[bt][:])
            nc.scalar.dma_start(out=out_d[s, bt * P:(bt + 1) * P, :], in_=o_sb[:])


_prog_cache = {}
_last_in_maps = None


def _get_program(key):
    if key not in _prog_cache:
        _prog_cache[key] = build_core_program(*key)
    return _prog_cache[key]


def kernel(x, weight_mu, weight_sigma, bias_mu, bias_sigma, eps_w, eps_b):
    global _last_in_maps
    x = np.ascontiguousarray(x, dtype=np.float32)
    weight_mu = np.ascontiguousarray(weight_mu, dtype=np.float32)
    weight_sigma = np.ascontiguousarray(weight_sigma, dtype=np.float32)
    bias_mu = np.ascontiguousarray(bias_mu, dtype=np.float32)
    bias_sigma = np.ascontiguousarray(bias_sigma, dtype=np.float32)
    eps_w = np.ascontiguousarray(eps_w, dtype=np.float32)
    eps_b = np.ascontiguousarray(eps_b, dtype=np.float32)

    nc = _get_program((S_PC, O_PC, IN, B))

    bf16 = mybir.dt.np(BF16)

    def pack2(a):          # (R, IN) -> (P, R//P, IN) bf16, row r = t*P + p
        return np.ascontiguousarray(
            a.reshape(-1, P, a.shape[-1]).transpose(1, 0, 2).astype(bf16))

    def pack3(a):          # (S, R, IN) -> (S, P, R//P, IN) bf16
        return np.ascontiguousarray(
            a.reshape(a.shape[0], -1, P, a.shape[-1]).transpose(0, 2, 1, 3)
            .astype(bf16))

    in_maps = []
    for c in range(N_CORES):
        sg, og = divmod(c, OUT_WAYS)
        s_lo, s_hi = sg * S_PC, (sg + 1) * S_PC
        o_lo, o_hi = og * O_PC, (og + 1) * O_PC
        in_maps.append({
            "x": pack3(x[s_lo:s_hi]),
            "eps_w": pack3(eps_w[s_lo:s_hi, o_lo:o_hi, :]),
            "mu": pack2(weight_mu[o_lo:o_hi]),
            "sigma": pack2(weight_sigma[o_lo:o_hi]),
            "bias_mu": bias_mu[o_lo:o_hi].reshape(1, O_PC),
            "bias_sigma": bias_sigma[o_lo:o_hi].reshape(1, O_PC),
            "eps_b": np.ascontiguousarray(eps_b[s_lo:s_hi, o_lo:o_hi]),
        })

    _last_in_maps = in_maps
    res = run_bass_kernel_spmd(nc, in_maps, core_ids=list(range(N_CORES)))

    out = np.empty((S, B, OUT), dtype=np.float32)
    for c in range(N_CORES):
        sg, og = divmod(c, OUT_WAYS)
        out[sg * S_PC:(sg + 1) * S_PC, :, og * O_PC:(og + 1) * O_PC] = res.results[c]["out"]
    return out

